# revision 1
# baseline (speedup 1.0000x reference)
"""Single-head attention kernel for Trainium2, SPMD over 8 NeuronCores.

Problem: x [4,4096,1024] f32 -> q/k/v = x@W+b (head 128) -> softmax(q k^T/sqrt(128)) @ v.
Sharding: core i handles batch i//2, query half i%2. Each core receives its
batch's full x with rows rotated so its 2048 queries are rows 0:2048 (key
order is irrelevant to softmax sums), so all cores run one identical program.

Perf notes (from NTFF traces on this hardware):
- fp32 matmul runs in LOW_HIGH 2-pass mode = 4 cycles/row; fp16 is 1 cyc/row
  with an 11-bit mantissa. All values here are O(10), so the whole compute
  path runs in fp16 with fp32 PSUM accumulation (measured ~4e-4 end-to-end).
- DMA-xbar transposes interleaved with regular DMAs thrash xbar_mode and
  serialize the DMA system; transposes run on the PE in transpose-mode
  (1 cyc/row for fp16) instead.
- PSUM accumulation groups: start=True clears the WHOLE bank, so each of the
  8 P@V accumulators gets its own bank-group; P is materialized in SBUF per
  query block and consumed qs-outer so only 4 accumulator banks are live.
- exp on ScalarE costs ~(N+352)/1.2ns per instruction; issued on [128,1024]
  PSUM spans to amortize. x f32->f16 downcasts also run on ScalarE (idle in
  phase 1); PSUM->SBUF copies run on VectorE.
- P@V appends a ones-column to V so the softmax denominator lands in PSUM
  column 128 of each accumulator for free.
"""

import sys

if "/opt/trn_rl_repo" not in sys.path:
    sys.path.insert(0, "/opt/trn_rl_repo")

import numpy as np

P = 128          # partitions
S = 4096         # sequence length
E = 1024         # n_embd
D = 128          # head size
SQ = 2048        # queries per core
SC = 512         # s-processing chunk (phase 1)
NSC = S // SC    # 8
NEC = E // P     # 8
NKT = S // P     # 32 key tiles
QBLK = 1024      # phase-2 query block (ACT instruction width)
NQB = SQ // QBLK # 2
SCALE = 1.0 / float(np.sqrt(D))

_CACHE = {}


def _build_nc():
    import concourse.mybir as mybir
    import concourse.tile as tile
    from concourse import bacc

    f32 = mybir.dt.float32
    f16 = mybir.dt.float16
    AF = mybir.ActivationFunctionType

    nc = bacc.Bacc(None, target_bir_lowering=False)
    x = nc.dram_tensor("x16", [S, E], f16, kind="ExternalInput")
    wq = nc.dram_tensor("wq", [E, D], f32, kind="ExternalInput")
    wk = nc.dram_tensor("wk", [E, D], f32, kind="ExternalInput")
    wv = nc.dram_tensor("wv", [E, D], f32, kind="ExternalInput")
    bq = nc.dram_tensor("bq", [D, 1], f32, kind="ExternalInput")
    bk = nc.dram_tensor("bk", [D, 1], f32, kind="ExternalInput")
    bv = nc.dram_tensor("bv", [D, 1], f32, kind="ExternalInput")
    ident = nc.dram_tensor("ident", [P, P], f32, kind="ExternalInput")
    out = nc.dram_tensor("out", [SQ, D], f32, kind="ExternalOutput")

    with tile.TileContext(nc) as tc:
        with tc.tile_pool(name="big", bufs=1) as bigp, \
             tc.tile_pool(name="op", bufs=4) as op, \
             tc.tile_pool(name="ppe", bufs=33) as ppe:

            phase1_pools = [
                tc.tile_pool(name="const", bufs=1),
                tc.tile_pool(name="xfp", bufs=10),
                tc.tile_pool(name="xtp", bufs=10),
                tc.tile_pool(name="vtmp", bufs=2),
            ]
            constp, xfp, xtp, vtmpp = [pl.__enter__() for pl in phase1_pools]
            # --- constants in SBUF ---
            # identity first: the very first PE transposes depend on it, and
            # DMAs issue in program order on the Sync queue.
            id_st = constp.tile([P, P], f32)
            nc.sync.dma_start(out=id_st, in_=ident[:, :])
            id16 = constp.tile([P, P], f16)
            nc.vector.tensor_copy(id16, id_st)
            # prefetch the first two s-chunks of x ahead of the weight DMAs
            pre_x = []
            for pi in range(8):
                x16 = xfp.tile([P, E], f16, tag="x16", name="x16")
                nc.sync.dma_start(out=x16, in_=x[pi * P:(pi + 1) * P, :])
                pre_x.append(x16)
            w16 = []
            for nm, w_dram in (("wq", wq), ("wk", wk), ("wv", wv)):
                w_st = constp.tile([P, E], f32, name=f"{nm}_st")
                for ec in range(NEC):
                    nc.sync.dma_start(out=w_st[:, ec * P:(ec + 1) * P],
                                      in_=w_dram[ec * P:(ec + 1) * P, :])
                w_sb = constp.tile([P, E], f16, name=f"{nm}16")
                nc.vector.tensor_copy(w_sb, w_st)
                w16.append(w_sb)
            wq_sb, wk_sb, wv_sb = w16
            bq_sb = constp.tile([P, 1], f32)
            bk_sb = constp.tile([P, 1], f32)
            bv_sb = constp.tile([P, 1], f32)
            nc.sync.dma_start(out=bq_sb, in_=bq[:, :])
            nc.sync.dma_start(out=bk_sb, in_=bk[:, :])
            nc.sync.dma_start(out=bv_sb, in_=bv[:, :])

            # persistent activations (all fp16)
            kT_sb = bigp.tile([P, S], f16)        # K^T  [d, s]
            qT_sb = bigp.tile([P, SQ], f16)       # Q^T  [d, q]
            v_all = bigp.tile([P, NKT, D + 1], f16)  # [k_local, kt, 128 V | ones]
            nc.vector.memset(v_all[:, :, D:D + 1], 1.0)

            # ---------------- phase 1: x load/downcast/transpose + QKV ----------------
            p0a = []

            def s_exp(sp_pool, p_pool, qb, kt, w=QBLK, qoff=0):
                sp = sp_pool.tile([P, w], f32, tag="sp", name="sp")
                for h in range(w // SC):
                    nc.tensor.matmul(sp[:, h * SC:(h + 1) * SC],
                                     kT_sb[:, kt * P:(kt + 1) * P],
                                     qT_sb[:, qb * QBLK + qoff + h * SC:
                                           qb * QBLK + qoff + (h + 1) * SC],
                                     start=True, stop=True)
                p_sb = p_pool.tile([P, w], f16, tag="p", name="p")
                nc.scalar.activation(p_sb, sp, AF.Exp, scale=SCALE)
                return p_sb

            with tc.tile_pool(name="tp_ps", bufs=2, space="PSUM") as tp_ps, \
                 tc.tile_pool(name="proj_ps", bufs=1, space="PSUM") as proj_ps, \
                 tc.tile_pool(name="vt_ps", bufs=1, space="PSUM") as vt_ps, \
                 tc.tile_pool(name="sp1_ps", bufs=2, space="PSUM") as sp1_ps:
                for sc in range(NSC):
                    x16s = []
                    for i in range(4):
                        if sc * 4 + i < 8:
                            x16 = pre_x[sc * 4 + i]
                        else:
                            x16 = xfp.tile([P, E], f16, tag="x16", name="x16")
                            nc.sync.dma_start(
                                out=x16, in_=x[sc * SC + i * P: sc * SC + (i + 1) * P, :])
                        x16s.append(x16)
                    xTs = []
                    for ec in range(NEC):
                        tp = tp_ps.tile([P, SC], f16, tag="tp", name="tp")
                        for i in range(4):
                            nc.tensor.transpose(tp[:, i * P:(i + 1) * P],
                                                x16s[i][:, ec * P:(ec + 1) * P],
                                                id16)
                        xT = xtp.tile([P, SC], f16, tag="xT", name="xT")
                        nc.vector.tensor_copy(xT, tp)
                        xTs.append(xT)
                    pk = proj_ps.tile([P, SC], f32, tag="pk", name="pk")
                    pv = proj_ps.tile([P, SC], f32, tag="pv", name="pv")
                    pq = proj_ps.tile([P, SC], f32, tag="pq", name="pq") if sc < NSC // 2 else None
                    for ec in range(NEC):
                        st, sp_ = (ec == 0), (ec == NEC - 1)
                        nc.tensor.matmul(pk, wk_sb[:, ec * P:(ec + 1) * P], xTs[ec],
                                         start=st, stop=sp_)
                        nc.tensor.matmul(pv, wv_sb[:, ec * P:(ec + 1) * P], xTs[ec],
                                         start=st, stop=sp_)
                        if pq is not None:
                            nc.tensor.matmul(pq, wq_sb[:, ec * P:(ec + 1) * P], xTs[ec],
                                             start=st, stop=sp_)
                    if sc >= NSC // 2:
                        for t in range(2):
                            kt0 = (sc - NSC // 2) * 4 + t
                            for h in range(2):
                                p0a.append(s_exp(sp1_ps, ppe, 0, kt0, w=SC, qoff=h * SC))
                    nc.vector.tensor_scalar_add(kT_sb[:, sc * SC:(sc + 1) * SC], pk, bk_sb)
                    if pq is not None:
                        nc.vector.tensor_scalar_add(qT_sb[:, sc * SC:(sc + 1) * SC], pq, bq_sb)
                    # V: bias add (f32 psum -> f16), PE transpose, pack into v_all
                    vtmp = vtmpp.tile([P, SC], f16, tag="vtmp", name="vtmp")
                    nc.vector.tensor_scalar_add(vtmp, pv, bv_sb)
                    vt = vt_ps.tile([P, SC], f16, tag="vt", name="vt")
                    for i in range(4):
                        nc.tensor.transpose(vt[:, i * P:(i + 1) * P],
                                            vtmp[:, i * P:(i + 1) * P],
                                            id16)
                    nc.vector.tensor_copy(
                        v_all[:, sc * 4:(sc + 1) * 4, 0:D],
                        vt[:, :].rearrange("p (b c) -> p b c", c=P))
                    if sc >= NSC // 2:
                        for t in range(2, 4):
                            kt0 = (sc - NSC // 2) * 4 + t
                            for h in range(2):
                                p0a.append(s_exp(sp1_ps, ppe, 0, kt0, w=SC, qoff=h * SC))

            # phase-1-only SBUF pools released: phase 2 needs the space for
            # 64 materialized P tiles (full cross-block overlap of S/exp and P@V)
            for pl in reversed(phase1_pools):
                pl.__exit__(None, None, None)

            # ---------------- phase 2: attention ----------------
            with tc.tile_pool(name="pp", bufs=50) as pp, \
                 tc.tile_pool(name="sp_ps", bufs=2, space="PSUM") as sp_ps, \
                 tc.tile_pool(name="acc_ps", bufs=4, space="PSUM") as acc_ps:
                p_tiles = {}
                for kt in range(NKT // 2, NKT):
                    p_tiles[(0, kt)] = s_exp(sp_ps, pp, 0, kt)
                for kt in range(NKT):
                    p_tiles[(1, kt)] = s_exp(sp_ps, pp, 1, kt)

                def plhs(qb, kt, qs):
                    if qb == 0 and kt < NKT // 2:
                        return p0a[2 * kt + qs // 4][:, (qs % 4) * P:(qs % 4 + 1) * P]
                    return p_tiles[(qb, kt)][:, qs * P:(qs + 1) * P]

                for qb in range(NQB):
                    for qs in range(QBLK // P):
                        acc = acc_ps.tile([P, D + 1], f32, tag="acc", name="acc")
                        for kt in range(NKT):
                            nc.tensor.matmul(acc, plhs(qb, kt, qs),
                                             v_all[:, kt, :],
                                             start=(kt == 0), stop=(kt == NKT - 1))
                        rec = op.tile([P, 1], f32, tag="rec", name="rec")
                        nc.vector.reciprocal(rec, acc[:, D:D + 1])
                        o_sb = op.tile([P, D], f32, tag="o", name="o")
                        nc.vector.tensor_scalar_mul(o_sb, acc[:, 0:D], rec)
                        q0 = (qb * (QBLK // P) + qs) * P
                        nc.sync.dma_start(out=out[q0:q0 + P, :], in_=o_sb)
    nc.finalize()
    return nc


def _get_nc():
    if "nc" not in _CACHE:
        _CACHE["nc"] = _build_nc()
    return _CACHE["nc"]


def _in_maps(x, Wq, bq, Wk, bk, Wv, bv):
    x = np.asarray(x, dtype=np.float32).astype(np.float16)
    shared = {
        "wq": np.ascontiguousarray(np.asarray(Wq, np.float32)),
        "wk": np.ascontiguousarray(np.asarray(Wk, np.float32)),
        "wv": np.ascontiguousarray(np.asarray(Wv, np.float32)),
        "bq": np.ascontiguousarray(np.asarray(bq, np.float32).reshape(D, 1)),
        "bk": np.ascontiguousarray(np.asarray(bk, np.float32).reshape(D, 1)),
        "bv": np.ascontiguousarray(np.asarray(bv, np.float32).reshape(D, 1)),
        "ident": np.eye(P, dtype=np.float32),
    }
    maps = []
    for core in range(8):
        b, h = core // 2, core % 2
        xb = x[b] if h == 0 else np.concatenate([x[b, SQ:], x[b, :SQ]], axis=0)
        maps.append({"x16": np.ascontiguousarray(xb), **shared})
    return maps


def _assemble(results):
    out = np.empty((4, S, D), dtype=np.float32)
    for core in range(8):
        b, h = core // 2, core % 2
        out[b, h * SQ:(h + 1) * SQ] = results[core]["out"]
    return out


def kernel(x, Wq, bq, Wk, bk, Wv, bv):
    from concourse.bass_utils import run_bass_kernel_spmd

    nc = _get_nc()
    res = run_bass_kernel_spmd(nc, _in_maps(x, Wq, bq, Wk, bk, Wv, bv),
                               core_ids=list(range(8)))
    return _assemble(res.results)



# revision 2
# speedup vs baseline: 1.2021x; 1.2021x over previous
"""Single-head attention kernel for Trainium2, SPMD over 8 NeuronCores.

Problem: x [4,4096,1024] f32 -> q/k/v = x@W+b (head 128) -> softmax(q k^T/sqrt(128)) @ v.
Sharding: core i handles batch i//2, query half i%2. Each core receives its
batch's x pre-transposed on the host to x^T [e, s] layout (and rotated so its
2048 queries are cols 0:2048; key order is irrelevant to softmax sums), so all
cores run one identical program and no on-chip transposes of x are needed.

Perf notes (from NTFF traces on this hardware):
- fp32 matmul runs in LOW_HIGH 2-pass mode = 4 cycles/row; fp16 is 1 cyc/row
  with an 11-bit mantissa. All values here are O(10), so the whole compute
  path runs in fp16 with fp32 PSUM accumulation (measured ~5e-4 end-to-end).
- Host supplies x^T chunked as [sc*128+p, ec*512+c] so each s-chunk is one
  [128, 4096] DMA with 8KB contiguous rows (one ~600ns issue slot per chunk).
- exp on ScalarE costs ~(N+352)/1.2ns per instruction; total exp work is
  ~55us/core, so all 32 qb=0 score tiles are computed+exp'd inside phase 1
  (starting once qT cols 0:1024 exist) to front-load ScalarE.
- PSUM accumulation groups: start=True clears the whole bank. Phase 1 uses
  pk/pv/pq (3 banks) + vt (1) + sp (2x2 banks); phase 2 sp (2x2) + acc (4).
- P@V appends a ones-column to V so the softmax denominator lands in PSUM
  column 128 of each accumulator for free.
"""

import sys

if "/opt/trn_rl_repo" not in sys.path:
    sys.path.insert(0, "/opt/trn_rl_repo")

import numpy as np

P = 128          # partitions
S = 4096         # sequence length
E = 1024         # n_embd
D = 128          # head size
SQ = 2048        # queries per core
SC = 512         # s-processing chunk (phase 1)
NSC = S // SC    # 8
NEC = E // P     # 8
NKT = S // P     # 32 key tiles
QBLK = 1024      # query block (ACT instruction width)
NQB = SQ // QBLK # 2
SCALE = 1.0 / float(np.sqrt(D))

_CACHE = {}


def _build_nc():
    import concourse.mybir as mybir
    import concourse.tile as tile
    from concourse import bacc

    f32 = mybir.dt.float32
    f16 = mybir.dt.float16
    AF = mybir.ActivationFunctionType

    nc = bacc.Bacc(None, target_bir_lowering=False)
    # xt[sc*128+p, ec*512+c] = x^T[ec*128+p, sc*512+c] (host pre-chunked)
    xt = nc.dram_tensor("xt", [S // SC * P, NEC * SC], f16, kind="ExternalInput")
    wq = nc.dram_tensor("wq", [E, D], f32, kind="ExternalInput")
    wk = nc.dram_tensor("wk", [E, D], f32, kind="ExternalInput")
    wv = nc.dram_tensor("wv", [E, D], f32, kind="ExternalInput")
    bq = nc.dram_tensor("bq", [D, 1], f32, kind="ExternalInput")
    bk = nc.dram_tensor("bk", [D, 1], f32, kind="ExternalInput")
    bv = nc.dram_tensor("bv", [D, 1], f32, kind="ExternalInput")
    ident = nc.dram_tensor("ident", [P, P], f16, kind="ExternalInput")
    out = nc.dram_tensor("out", [SQ, D], f32, kind="ExternalOutput")

    with tile.TileContext(nc) as tc:
        with tc.tile_pool(name="big", bufs=1) as bigp, \
             tc.tile_pool(name="op", bufs=4) as op, \
             tc.tile_pool(name="p0", bufs=NKT) as p0pool:

            phase1_pools = [
                tc.tile_pool(name="const", bufs=1),
                tc.tile_pool(name="xtp", bufs=4),
                tc.tile_pool(name="vtmp", bufs=2),
            ]
            constp, xtp, vtmpp = [pl.__enter__() for pl in phase1_pools]
            # x chunk 0 first: everything downstream waits on it, and DMAs
            # issue in program order on the Sync queue.
            xts = []
            x0 = xtp.tile([P, E * 4], f16, tag="xt", name="xt")
            nc.sync.dma_start(out=x0, in_=xt[0:P, :])
            xts.append(x0)
            id16 = constp.tile([P, P], f16)
            nc.sync.dma_start(out=id16, in_=ident[:, :])
            x1 = xtp.tile([P, E * 4], f16, tag="xt", name="xt")
            nc.sync.dma_start(out=x1, in_=xt[P:2 * P, :])
            xts.append(x1)
            w16 = []
            for nm, w_dram in (("wq", wq), ("wk", wk), ("wv", wv)):
                w_st = constp.tile([P, E], f32, name=f"{nm}_st")
                for ec in range(NEC):
                    nc.sync.dma_start(out=w_st[:, ec * P:(ec + 1) * P],
                                      in_=w_dram[ec * P:(ec + 1) * P, :])
                w_sb = constp.tile([P, E], f16, name=f"{nm}16")
                nc.vector.tensor_copy(w_sb, w_st)
                w16.append(w_sb)
            wq_sb, wk_sb, wv_sb = w16
            bq_sb = constp.tile([P, 1], f32)
            bk_sb = constp.tile([P, 1], f32)
            bv_sb = constp.tile([P, 1], f32)
            nc.sync.dma_start(out=bq_sb, in_=bq[:, :])
            nc.sync.dma_start(out=bk_sb, in_=bk[:, :])
            nc.sync.dma_start(out=bv_sb, in_=bv[:, :])

            # persistent activations (all fp16)
            kT_sb = bigp.tile([P, S], f16)        # K^T  [d, s]
            qT_sb = bigp.tile([P, SQ], f16)       # Q^T  [d, q]
            v_all = bigp.tile([P, NKT, D + 1], f16)  # [k_local, kt, 128 V | ones]
            nc.vector.memset(v_all[:, :, D:D + 1], 1.0)

            def s_exp(sp_pool, p_pool, qb, kt, w=QBLK, qoff=0):
                sp = sp_pool.tile([P, w], f32, tag="sp", name="sp")
                for h in range(w // SC):
                    nc.tensor.matmul(sp[:, h * SC:(h + 1) * SC],
                                     kT_sb[:, kt * P:(kt + 1) * P],
                                     qT_sb[:, qb * QBLK + qoff + h * SC:
                                           qb * QBLK + qoff + (h + 1) * SC],
                                     start=True, stop=True)
                p_sb = p_pool.tile([P, w], f16, tag="p", name="p")
                nc.scalar.activation(p_sb, sp, AF.Exp, scale=SCALE)
                return p_sb

            # qb=0 score tiles to interleave after each s-chunk's projections:
            # chunk sc makes kt 4sc..4sc+3 available; qb0 needs qT from chunks
            # 0-1, so issuing starts at sc=2 and drains the backlog by sc=7.
            p1_kts = {2: range(0, 4), 3: range(4, 10), 4: range(10, 16),
                      5: range(16, 22), 6: range(22, 28), 7: range(28, 32)}
            p0_tiles = {}

            # ---------------- phase 1: QKV + all qb=0 scores ----------------
            with tc.tile_pool(name="proj_ps", bufs=1, space="PSUM") as proj_ps, \
                 tc.tile_pool(name="vt_ps", bufs=1, space="PSUM") as vt_ps, \
                 tc.tile_pool(name="sp1_ps", bufs=2, space="PSUM") as sp1_ps:
                for sc in range(NSC):
                    if sc < 2:
                        x16 = xts[sc]
                    else:
                        x16 = xtp.tile([P, E * 4], f16, tag="xt", name="xt")
                        nc.sync.dma_start(out=x16, in_=xt[sc * P:(sc + 1) * P, :])
                    pk = proj_ps.tile([P, SC], f32, tag="pk", name="pk")
                    pv = proj_ps.tile([P, SC], f32, tag="pv", name="pv")
                    pq = proj_ps.tile([P, SC], f32, tag="pq", name="pq") if sc < NSC // 2 else None
                    for ec in range(NEC):
                        st, sp_ = (ec == 0), (ec == NEC - 1)
                        rhs = x16[:, ec * SC:(ec + 1) * SC]
                        nc.tensor.matmul(pk, wk_sb[:, ec * P:(ec + 1) * P], rhs,
                                         start=st, stop=sp_)
                        nc.tensor.matmul(pv, wv_sb[:, ec * P:(ec + 1) * P], rhs,
                                         start=st, stop=sp_)
                        if pq is not None:
                            nc.tensor.matmul(pq, wq_sb[:, ec * P:(ec + 1) * P], rhs,
                                             start=st, stop=sp_)
                    nc.vector.tensor_scalar_add(kT_sb[:, sc * SC:(sc + 1) * SC], pk, bk_sb)
                    if pq is not None:
                        nc.vector.tensor_scalar_add(qT_sb[:, sc * SC:(sc + 1) * SC], pq, bq_sb)
                    # V: bias add (f32 psum -> f16), PE transpose, pack into v_all
                    vtmp = vtmpp.tile([P, SC], f16, tag="vtmp", name="vtmp")
                    nc.vector.tensor_scalar_add(vtmp, pv, bv_sb)
                    vt = vt_ps.tile([P, SC], f16, tag="vt", name="vt")
                    for i in range(4):
                        nc.tensor.transpose(vt[:, i * P:(i + 1) * P],
                                            vtmp[:, i * P:(i + 1) * P],
                                            id16)
                    nc.vector.tensor_copy(
                        v_all[:, sc * 4:(sc + 1) * 4, 0:D],
                        vt[:, :].rearrange("p (b c) -> p b c", c=P))
                    for kt in p1_kts.get(sc, ()):
                        p0_tiles[kt] = s_exp(sp1_ps, p0pool, 0, kt)

            # phase-1-only SBUF pools released: phase 2 needs the space for
            # the 32 qb=1 P tiles (full cross-block overlap of S/exp and P@V)
            for pl in reversed(phase1_pools):
                pl.__exit__(None, None, None)

            # ---------------- phase 2: qb=1 scores + both P@V sweeps ----------------
            with tc.tile_pool(name="pp", bufs=NKT) as pp, \
                 tc.tile_pool(name="sp_ps", bufs=2, space="PSUM") as sp_ps, \
                 tc.tile_pool(name="acc_ps", bufs=4, space="PSUM") as acc_ps:
                p_tiles = {(0, kt): t for kt, t in p0_tiles.items()}
                for kt in range(NKT):
                    p_tiles[(1, kt)] = s_exp(sp_ps, pp, 1, kt)

                for qb in range(NQB):
                    for qs in range(QBLK // P):
                        acc = acc_ps.tile([P, D + 1], f32, tag="acc", name="acc")
                        for kt in range(NKT):
                            nc.tensor.matmul(acc,
                                             p_tiles[(qb, kt)][:, qs * P:(qs + 1) * P],
                                             v_all[:, kt, :],
                                             start=(kt == 0), stop=(kt == NKT - 1))
                        rec = op.tile([P, 1], f32, tag="rec", name="rec")
                        nc.vector.reciprocal(rec, acc[:, D:D + 1])
                        o_sb = op.tile([P, D], f32, tag="o", name="o")
                        nc.vector.tensor_scalar_mul(o_sb, acc[:, 0:D], rec)
                        q0 = (qb * (QBLK // P) + qs) * P
                        nc.sync.dma_start(out=out[q0:q0 + P, :], in_=o_sb)
    nc.finalize()
    return nc


def _get_nc():
    if "nc" not in _CACHE:
        _CACHE["nc"] = _build_nc()
    return _CACHE["nc"]


def _in_maps(x, Wq, bq, Wk, bk, Wv, bv):
    x = np.asarray(x, dtype=np.float32).astype(np.float16)
    shared = {
        "wq": np.ascontiguousarray(np.asarray(Wq, np.float32)),
        "wk": np.ascontiguousarray(np.asarray(Wk, np.float32)),
        "wv": np.ascontiguousarray(np.asarray(Wv, np.float32)),
        "bq": np.ascontiguousarray(np.asarray(bq, np.float32).reshape(D, 1)),
        "bk": np.ascontiguousarray(np.asarray(bk, np.float32).reshape(D, 1)),
        "bv": np.ascontiguousarray(np.asarray(bv, np.float32).reshape(D, 1)),
        "ident": np.eye(P, dtype=np.float16),
    }
    maps = []
    for core in range(8):
        b, h = core // 2, core % 2
        xb = x[b] if h == 0 else np.concatenate([x[b, SQ:], x[b, :SQ]], axis=0)
        # [s, e] -> x^T [e, s] -> chunk layout [sc*128+p, ec*512+c]
        xT = xb.T  # [E, S]
        y = xT.reshape(NEC, P, NSC, SC).transpose(2, 1, 0, 3).reshape(NSC * P, NEC * SC)
        maps.append({"xt": np.ascontiguousarray(y), **shared})
    return maps


def _assemble(results):
    out = np.empty((4, S, D), dtype=np.float32)
    for core in range(8):
        b, h = core // 2, core % 2
        out[b, h * SQ:(h + 1) * SQ] = results[core]["out"]
    return out


def kernel(x, Wq, bq, Wk, bk, Wv, bv):
    from concourse.bass_utils import run_bass_kernel_spmd

    nc = _get_nc()
    res = run_bass_kernel_spmd(nc, _in_maps(x, Wq, bq, Wk, bk, Wv, bv),
                               core_ids=list(range(8)))
    return _assemble(res.results)


# revision 7
# speedup vs baseline: 1.9799x; 1.6471x over previous
"""Single-head attention kernel for Trainium2, SPMD over 8 NeuronCores.

Problem: x [4,4096,1024] f32 -> q/k/v = x@W+b (head 128) -> softmax(q k^T/sqrt(128)) @ v.
Sharding: core i handles batch i//2, query half i%2. Each core receives its
batch's x pre-transposed on the host to x^T [e, s] layout (and rotated so its
2048 queries are cols 0:2048; key order is irrelevant to softmax sums), so all
cores run one identical program and no on-chip transposes of x are needed.

Perf notes (from NTFF traces on this hardware):
- fp32 matmul runs in LOW_HIGH 2-pass mode = 4 cycles/row; fp16 is 1 cyc/row
  with an 11-bit mantissa. All values here are O(10), so the whole compute
  path runs in fp16 with fp32 PSUM accumulation (measured ~5e-4 end-to-end).
- Host supplies x^T chunked as [sc*128+p, ec*512+c] so each s-chunk is one
  [128, 4096] DMA with 8KB contiguous rows (one ~600ns issue slot per chunk).
- exp on ScalarE costs ~(N+352)/1.2ns per instruction; total exp work is
  ~55us/core, so all 32 qb=0 score tiles are computed+exp'd inside phase 1
  (starting once qT cols 0:1024 exist) to front-load ScalarE.
- PSUM accumulation groups: start=True clears the whole bank. Phase 1 uses
  pk/pv/pq (3 banks) + vt (1) + sp (2x2 banks); phase 2 sp (2x2) + acc (4).
- P@V appends a ones-column to V so the softmax denominator lands in PSUM
  column 128 of each accumulator for free.
"""

import sys

if "/opt/trn_rl_repo" not in sys.path:
    sys.path.insert(0, "/opt/trn_rl_repo")

import numpy as np

P = 128          # partitions
S = 4096         # sequence length
E = 1024         # n_embd
D = 128          # head size
SQ = 2048        # queries per core
SC = 512         # s-processing chunk (phase 1)
NSC = S // SC    # 8
NEC = E // P     # 8
NKT = S // P     # 32 key tiles
QBLK = 1024      # query block (ACT instruction width)
NQB = SQ // QBLK # 2
SCALE = 1.0 / float(np.sqrt(D))

_CACHE = {}


def _build_nc():
    import concourse.mybir as mybir
    import concourse.tile as tile
    from concourse import bacc

    f32 = mybir.dt.float32
    f16 = mybir.dt.float16
    AF = mybir.ActivationFunctionType

    nc = bacc.Bacc(None, target_bir_lowering=False)
    # xt[sc*128+p, ec*512+c] = x^T[ec*128+p, sc*512+c] (host pre-chunked)
    xt = nc.dram_tensor("xt", [S // SC * P, NEC * SC], f16, kind="ExternalInput")
    # weights host-prepacked to [p, ec*128+c] = W[ec*128+p, c], f16
    wq = nc.dram_tensor("wqp", [P, E], f16, kind="ExternalInput")
    wk = nc.dram_tensor("wkp", [P, E], f16, kind="ExternalInput")
    wv = nc.dram_tensor("wvp", [P, E], f16, kind="ExternalInput")
    bias = nc.dram_tensor("bias", [P, 3], f32, kind="ExternalInput")  # [bk|bv|bq]
    ident = nc.dram_tensor("ident", [P, P], f16, kind="ExternalInput")
    out = nc.dram_tensor("out", [SQ, D], f32, kind="ExternalOutput")

    with tile.TileContext(nc) as tc:
        with tc.tile_pool(name="big", bufs=1) as bigp, \
             tc.tile_pool(name="op", bufs=4) as op, \
             tc.tile_pool(name="p0", bufs=NKT) as p0pool:

            phase1_pools = [
                tc.tile_pool(name="const", bufs=1),
                tc.tile_pool(name="xtp", bufs=4),
                tc.tile_pool(name="vtmp", bufs=2),
            ]
            constp, xtp, vtmpp = [pl.__enter__() for pl in phase1_pools]
            # DMA issue order matters: the Sync queue dispatches ~1 descriptor
            # per 600ns, so the first projection's operands go first.
            xts = []
            x0 = xtp.tile([P, E * 4], f16, tag="xt", name="xt")
            nc.sync.dma_start(out=x0[:, 0:E * 2], in_=xt[0:P, 0:E * 2])
            wk_sb = constp.tile([P, E], f16, name="wk16")
            nc.sync.dma_start(out=wk_sb, in_=wk[:, :])
            nc.sync.dma_start(out=x0[:, E * 2:E * 4], in_=xt[0:P, E * 2:E * 4])
            wv_sb = constp.tile([P, E], f16, name="wv16")
            nc.sync.dma_start(out=wv_sb, in_=wv[:, :])
            wq_sb = constp.tile([P, E], f16, name="wq16")
            nc.sync.dma_start(out=wq_sb, in_=wq[:, :])
            bias_sb = constp.tile([P, 3], f32, name="bias")
            nc.sync.dma_start(out=bias_sb, in_=bias[:, :])
            id16 = constp.tile([P, P], f16)
            nc.sync.dma_start(out=id16, in_=ident[:, :])
            xts.append(x0)
            x1 = xtp.tile([P, E * 4], f16, tag="xt", name="xt")
            nc.sync.dma_start(out=x1, in_=xt[P:2 * P, :])
            xts.append(x1)
            bk_sb = bias_sb[:, 0:1]
            bv_sb = bias_sb[:, 1:2]
            bq_sb = bias_sb[:, 2:3]

            # persistent activations (all fp16)
            kT_sb = bigp.tile([P, S], f16)        # K^T  [d, s]
            qT_sb = bigp.tile([P, SQ], f16)       # Q^T  [d, q]
            v_all = bigp.tile([P, NKT, D + 1], f16)  # [k_local, kt, 128 V | ones]
            nc.vector.memset(v_all[:, :, D:D + 1], 1.0)

            def s_exp(sp_pool, p_pool, qb, kt, w=QBLK, qoff=0):
                sp = sp_pool.tile([P, w], f32, tag="sp", name="sp")
                for h in range(w // SC):
                    nc.tensor.matmul(sp[:, h * SC:(h + 1) * SC],
                                     kT_sb[:, kt * P:(kt + 1) * P],
                                     qT_sb[:, qb * QBLK + qoff + h * SC:
                                           qb * QBLK + qoff + (h + 1) * SC],
                                     start=True, stop=True)
                p_sb = p_pool.tile([P, w], f16, tag="p", name="p")
                nc.scalar.activation(p_sb, sp, AF.Exp, scale=SCALE)
                return p_sb

            # qb=0 score tiles to interleave after each s-chunk's projections:
            # chunk sc makes kt 4sc..4sc+3 available; qb0 needs qT from chunks
            # 0-1, so issuing starts at sc=1 and drains the backlog by sc=7.
            p1_kts = {1: range(0, 4), 2: range(4, 10), 3: range(10, 16),
                      4: range(16, 20), 5: range(20, 24), 6: range(24, 28),
                      7: range(28, 32)}
            p0_tiles = {}

            # ---------------- phase 1: QKV + all qb=0 scores ----------------
            with tc.tile_pool(name="proj_ps", bufs=1, space="PSUM") as proj_ps, \
                 tc.tile_pool(name="vt_ps", bufs=1, space="PSUM") as vt_ps, \
                 tc.tile_pool(name="sp1_ps", bufs=2, space="PSUM") as sp1_ps:
                for sc in range(NSC):
                    if sc < 2:
                        x16 = xts[sc]
                    else:
                        x16 = xtp.tile([P, E * 4], f16, tag="xt", name="xt")
                        nc.sync.dma_start(out=x16, in_=xt[sc * P:(sc + 1) * P, :])
                    # sequential K, V, Q accumulation: kT's bias-add (and so
                    # the interleaved scores) fire 2x earlier than with
                    # per-ec interleaving of all three projections.
                    pk = proj_ps.tile([P, SC], f32, tag="pk", name="pk")
                    pv = proj_ps.tile([P, SC], f32, tag="pv", name="pv")
                    pq = proj_ps.tile([P, SC], f32, tag="pq", name="pq") if sc < NSC // 2 else None
                    for dst, w_sb in ((pk, wk_sb), (pv, wv_sb), (pq, wq_sb)):
                        if dst is None:
                            continue
                        for ec in range(NEC):
                            nc.tensor.matmul(dst, w_sb[:, ec * P:(ec + 1) * P],
                                             x16[:, ec * SC:(ec + 1) * SC],
                                             start=(ec == 0), stop=(ec == NEC - 1))
                    nc.vector.tensor_scalar_add(kT_sb[:, sc * SC:(sc + 1) * SC], pk, bk_sb)
                    if pq is not None:
                        nc.vector.tensor_scalar_add(qT_sb[:, sc * SC:(sc + 1) * SC], pq, bq_sb)
                    # V: bias add (f32 psum -> f16), PE transpose, pack into v_all
                    vtmp = vtmpp.tile([P, SC], f16, tag="vtmp", name="vtmp")
                    nc.vector.tensor_scalar_add(vtmp, pv, bv_sb)
                    vt = vt_ps.tile([P, SC], f16, tag="vt", name="vt")
                    for i in range(4):
                        nc.tensor.transpose(vt[:, i * P:(i + 1) * P],
                                            vtmp[:, i * P:(i + 1) * P],
                                            id16)
                    nc.vector.tensor_copy(
                        v_all[:, sc * 4:(sc + 1) * 4, 0:D],
                        vt[:, :].rearrange("p (b c) -> p b c", c=P))
                    for kt in p1_kts.get(sc, ()):
                        p0_tiles[kt] = s_exp(sp1_ps, p0pool, 0, kt)

            # phase-1-only SBUF pools released: phase 2 needs the space for
            # the 32 qb=1 P tiles (full cross-block overlap of S/exp and P@V)
            for pl in reversed(phase1_pools):
                pl.__exit__(None, None, None)

            # ---------------- phase 2: qb=1 scores + both P@V sweeps ----------------
            with tc.tile_pool(name="pp", bufs=NKT) as pp, \
                 tc.tile_pool(name="sp_ps", bufs=2, space="PSUM") as sp_ps, \
                 tc.tile_pool(name="acc_ps", bufs=4, space="PSUM") as acc_ps:
                p_tiles = {(0, kt): t for kt, t in p0_tiles.items()}
                for kt in range(NKT):
                    p_tiles[(1, kt)] = s_exp(sp_ps, pp, 1, kt)

                for qb in range(NQB):
                    for qs in range(QBLK // P):
                        acc = acc_ps.tile([P, D + 1], f32, tag="acc", name="acc")
                        for kt in range(NKT):
                            nc.tensor.matmul(acc,
                                             p_tiles[(qb, kt)][:, qs * P:(qs + 1) * P],
                                             v_all[:, kt, :],
                                             start=(kt == 0), stop=(kt == NKT - 1))
                        rec = op.tile([P, 1], f32, tag="rec", name="rec")
                        nc.vector.reciprocal(rec, acc[:, D:D + 1])
                        o_sb = op.tile([P, D], f32, tag="o", name="o")
                        nc.vector.tensor_scalar_mul(o_sb, acc[:, 0:D], rec)
                        q0 = (qb * (QBLK // P) + qs) * P
                        nc.sync.dma_start(out=out[q0:q0 + P, :], in_=o_sb)
    nc.finalize()
    return nc


def _get_nc():
    if "nc" not in _CACHE:
        _CACHE["nc"] = _build_nc()
    return _CACHE["nc"]


def _prepack_w(w):
    # [E, D] f32 -> [p, ec*128+c] = W[ec*128+p, c], f16
    w = np.asarray(w, np.float32).astype(np.float16)
    return np.ascontiguousarray(
        w.reshape(NEC, P, D).transpose(1, 0, 2).reshape(P, E))


def _in_maps(x, Wq, bq, Wk, bk, Wv, bv):
    x = np.asarray(x, dtype=np.float32).astype(np.float16)
    bias = np.stack([np.asarray(b, np.float32).reshape(D) for b in (bk, bv, bq)],
                    axis=1)
    shared = {
        "wqp": _prepack_w(Wq),
        "wkp": _prepack_w(Wk),
        "wvp": _prepack_w(Wv),
        "bias": np.ascontiguousarray(bias),
        "ident": np.eye(P, dtype=np.float16),
    }
    maps = []
    for core in range(8):
        b, h = core // 2, core % 2
        xb = x[b] if h == 0 else np.concatenate([x[b, SQ:], x[b, :SQ]], axis=0)
        # [s, e] -> x^T [e, s] -> chunk layout [sc*128+p, ec*512+c]
        xT = xb.T  # [E, S]
        y = xT.reshape(NEC, P, NSC, SC).transpose(2, 1, 0, 3).reshape(NSC * P, NEC * SC)
        maps.append({"xt": np.ascontiguousarray(y), **shared})
    return maps


def _assemble(results):
    out = np.empty((4, S, D), dtype=np.float32)
    for core in range(8):
        b, h = core // 2, core % 2
        out[b, h * SQ:(h + 1) * SQ] = results[core]["out"]
    return out


def kernel(x, Wq, bq, Wk, bk, Wv, bv):
    from concourse.bass_utils import run_bass_kernel_spmd

    nc = _get_nc()
    res = run_bass_kernel_spmd(nc, _in_maps(x, Wq, bq, Wk, bk, Wv, bv),
                               core_ids=list(range(8)))
    return _assemble(res.results)
